# revision 2
# baseline (speedup 1.0000x reference)
# Block-circulant linear kernel for Trainium2 (Bass/Tile), 8-core SPMD.
#
# y[b, 16m+p] = sum_{n,q} blocks[(m-n)%512, p, q] * x[b, 16n+q]
#
# Strategy: shard the output block axis m across 8 cores (64 block-rows each).
# Per core, store a doubled+shifted "BIGQ" layout of blocks in SBUF:
#     BIGQ[(ni,q), u*16+p] = blocks[(m0 + u - ni) % 512, p, q]
# so that EVERY 128x128 weight tile of the implied 8192x8192 circulant matrix
# is a contiguous 128-column slice of BIGQ (the circulant gather becomes pure
# addressing). All (m_tile t, n_chunk c) pairs with the same diagonal offset
# d = t - c share one stationary tile, so the whole per-core compute is 71
# accumulating matmuls into a single PSUM bank [128 mp, 8 t x 32 b].
#
# The xt layout is reversed (c' = 63 - c) and the psum t axis flipped
# (t' = 7 - t) so both the weight stream (BIGQ u ascending) and the moving
# stream (xt c' ascending) are consumed in DMA arrival order.
#
# v2 (this file) vs v1 baseline (28.6us):
#  - 2 fp32 warmup matmuls instead of 4: the old warmup ended ~2.3us after the
#    first bigq chunk's completion semaphore, delaying the real stream.
#  - input DMA schedule: 6 bigq chunks (first 128 cols of BIGQ are dead — u0
#    starts at 8) + 2 xt halves, xt half 2 issued 4th so the mid-stream
#    diagonals (i>=32) never wait on it; 8 input DMAs total keeps every DMA on
#    a distinct completion semaphore (no reuse-serialization of late chunks).
#  - split output: psum cols 128:256 (t'=4..7) take their last matmul write at
#    diagonal i=66, so they are copied+DMAd while diagonals 67-70 still run;
#    only 64KB remains after the last matmul.
import os
import numpy as np

B = 32
NB = 512          # number of 16x16 blocks
NCORES = 8
MBLK = NB // NCORES   # 64 output block-rows per core
W = 576               # BIGQ window width (in u units of 16 columns)
ND = 71               # diagonal offsets d in [-63, 7]

DTYPE = "float16"
NWARM = int(os.environ.get("KNWARM", "2"))
SPLIT_OUT = os.environ.get("KSPLIT_OUT", "1") == "1"

_cached = {}
_last_results = None  # BassKernelResults of the most recent run (for profiling)


def _np_dtype(name):
    if name == "bfloat16":
        import ml_dtypes

        return ml_dtypes.bfloat16
    if name == "float16":
        return np.float16
    return np.float32


def _split_dt(dt_name):
    """'wt:mv' -> (weight dtype, moving dtype); single name -> same both."""
    if ":" in dt_name:
        wt, mv = dt_name.split(":")
        return wt, mv
    return dt_name, dt_name


def _build_program(dt_name):
    import concourse.bacc as bacc
    import concourse.mybir as mybir
    import concourse.tile as tile

    wt_name, mv_name = _split_dt(dt_name)
    wdt = getattr(mybir.dt, wt_name)
    mdt = getattr(mybir.dt, mv_name)
    f32 = mybir.dt.float32

    # Bacc (not plain Bass): its compile() pipeline splits multi-wait
    # instructions into EventSemaphore preludes (HW allows 1 wait/inst).
    nc = bacc.Bacc("TRN2", target_bir_lowering=False, debug=False, num_devices=NCORES)
    xt_d = nc.declare_dram_parameter("xt", [128, 2048], mdt, isOutput=False)
    bq_d = nc.declare_dram_parameter("bigq", [128, W * 16], wdt, isOutput=False)
    out_d = nc.declare_dram_parameter("out", [128, 256], f32, isOutput=True)

    with tile.TileContext(nc) as tc:
        with (
            tc.tile_pool(name="data", bufs=1) as data_pool,
            tc.tile_pool(name="psum", bufs=1, space="PSUM") as psum_pool,
        ):
            xt = data_pool.tile([128, 2048], mdt)
            bq = data_pool.tile([128, W * 16], wdt)
            out_sb = data_pool.tile([128, 256], f32)
            warm_sb = data_pool.tile([128, 256], f32)
            acc = psum_pool.tile([128, 256], f32)
            warm_ps = psum_pool.tile([128, 256], f32)

            # Input DMA schedule, in matmul consumption order. BIGQ cols
            # [0,128) are never read (stationary tiles start at u0=8), so the
            # bigq stream covers [128, 9216) in 6 chunks. xt half 2 is only
            # needed from diagonal i=32, but issue it early (4th) so its
            # completion semaphore fires long before the stream gets there.
            # Alternate the two HWDGE rings (sync=SP, scalar=ACT).
            order = [
                (xt, xt_d, 0, 1024),
                (bq, bq_d, 128, 1664),
                (bq, bq_d, 1664, 3200),
                (xt, xt_d, 1024, 2048),
                (bq, bq_d, 3200, 4736),
                (bq, bq_d, 4736, 6272),
                (bq, bq_d, 6272, 7808),
                (bq, bq_d, 7808, 9216),
            ]
            eng = [nc.sync, nc.scalar]
            for k, (tile_, dram_, lo, hi) in enumerate(order):
                eng[k % 2].dma_start(tile_[:, lo:hi], dram_[:, lo:hi])

            # PE warm-up while the first chunks stream in: the HAM clock gate
            # flips to 2.4GHz after ~3.4us of sustained PE activity measured
            # from warmup start, so what matters is starting the PE early;
            # the warmup only needs to bridge until the first chunk's
            # completion semaphore (~2us), not the full 3.4us. Each fp32
            # N=256 matmul is ~850ns cold.
            # memset on DVE, not gpsimd: DVE clears its engine preamble ~2us
            # earlier, so the warm-up (which waits on this) starts sooner.
            if NWARM:
                nc.vector.memset(warm_sb[:], 0.0)
            for wi in range(NWARM):
                nc.tensor.matmul(
                    warm_ps[:], warm_sb[:, 0:128], warm_sb[:],
                    start=(wi == 0), stop=(wi == NWARM - 1),
                )

            # d = t - c diagonal; stationary tile = BIGQ columns [16*u0, 16*u0+128)
            # with u0 = 8*i + 8 for i = 0..70 (d = i - 63).
            for i in range(ND):
                d = i - 63
                u0 = 8 * i + 8
                t_lo = max(0, d)
                t_hi = min(7, 63 + d)
                nt = t_hi - t_lo + 1
                tp_lo = 7 - t_hi           # flipped psum tile index
                cp_lo = 63 + d - t_hi      # reversed xt chunk index
                nc.tensor.matmul(
                    acc[:, 32 * tp_lo: 32 * (tp_lo + nt)],
                    bq[:, 16 * u0: 16 * u0 + 128],
                    xt[:, 32 * cp_lo: 32 * (cp_lo + nt)],
                    start=(i == 0),   # clears the whole PSUM bank
                    stop=(i == ND - 1),
                    skip_group_check=True,
                )

            if SPLIT_OUT:
                # psum cols 128:256 (t'=4..7) are final after diagonal 66
                # (its matmul writes cols 0:160; diagonals 67-70 write within
                # cols 0:128). Copy + store them while the tail runs.
                nc.vector.tensor_copy(out_sb[:, 128:256], acc[:, 128:256])
                nc.sync.dma_start(out_d[:, 128:256], out_sb[:, 128:256])
                nc.vector.tensor_copy(out_sb[:, 0:128], acc[:, 0:128])
                nc.sync.dma_start(out_d[:, 0:128], out_sb[:, 0:128])
            else:
                nc.vector.tensor_copy(out_sb[:], acc[:])
                nc.sync.dma_start(out_d[:], out_sb[:])
    nc.compile()
    return nc


def _get_program(dt_name):
    key = (dt_name, NWARM, SPLIT_OUT)
    if key not in _cached:
        _cached[key] = _build_program(dt_name)
    return _cached[key]


def _prep_inputs(x, blocks, dt_name):
    """Host-side layout prep (pure numpy reshuffles of the small inputs)."""
    x = np.ascontiguousarray(np.asarray(x), dtype=np.float32)
    blocks = np.ascontiguousarray(np.asarray(blocks), dtype=np.float32)
    # xt[(ni*16+q), c*32+b] = x[b, 128c + 16ni + q], then reverse c (c'=63-c)
    xt = x.T.reshape(64, 128, 32).transpose(1, 0, 2)[:, ::-1, :].reshape(128, 2048)
    xt = np.ascontiguousarray(xt)
    u = np.arange(W)
    ni = np.arange(8)
    wt_name, mv_name = _split_dt(dt_name)
    np_w, np_m = _np_dtype(wt_name), _np_dtype(mv_name)
    xt_c = np.ascontiguousarray(xt.astype(np_m))
    in_maps = []
    for k in range(NCORES):
        m0 = k * MBLK
        idx = (m0 + u[None, :] - ni[:, None]) % NB        # [8, W]
        bigq = blocks[idx]                                 # [8, W, p, q]
        bigq = bigq.transpose(0, 3, 1, 2).reshape(128, W * 16)  # [(ni,q), (u,p)]
        in_maps.append(
            {"xt": xt_c, "bigq": np.ascontiguousarray(bigq.astype(np_w))}
        )
    return in_maps


def _assemble(results):
    y = np.empty((B, NB * 16), dtype=np.float32)
    for k in range(NCORES):
        o = np.asarray(results[k]["out"])  # [128 (mi,p), 256 (t',b)], t = 7-t'
        y[:, 1024 * k: 1024 * (k + 1)] = (
            o.reshape(128, 8, 32)[:, ::-1, :].transpose(2, 1, 0).reshape(32, 1024)
        )
    return y


def kernel(x, blocks):
    global _last_results
    from concourse.bass_utils import run_bass_kernel_spmd

    nc = _get_program(DTYPE)
    in_maps = _prep_inputs(x, blocks, DTYPE)
    res = run_bass_kernel_spmd(nc, in_maps, list(range(NCORES)))
    _last_results = res
    return _assemble(res.results)


# revision 5
# speedup vs baseline: 1.1290x; 1.1290x over previous
# Block-circulant linear kernel for Trainium2 (Bass/Tile), 8-core SPMD.
#
# y[b, 16m+p] = sum_{n,q} blocks[(m-n)%512, p, q] * x[b, 16n+q]
#
# Strategy: shard the output block axis m across 8 cores (64 block-rows each).
# Per core, store a doubled+shifted "BIGQ" layout of blocks in SBUF:
#     BIGQ[(ni,q), u*16+p] = blocks[(m0 + u - ni) % 512, p, q]
# so that EVERY 128x128 weight tile of the implied 8192x8192 circulant matrix
# is a contiguous 128-column slice of BIGQ (the circulant gather becomes pure
# addressing). All (m_tile t, n_chunk c) pairs with the same diagonal offset
# d = t - c share one stationary tile, so the whole per-core compute is 71
# accumulating matmuls into a single PSUM bank [128 mp, 8 t x 32 b].
#
# The xt layout is reversed (c' = 63 - c) and the psum t axis flipped
# (t' = 7 - t) so both the weight stream (BIGQ u ascending) and the moving
# stream (xt c' ascending) are consumed in DMA arrival order.
#
# v2 (this file) vs v1 baseline (28.6us):
#  - 2 fp32 warmup matmuls instead of 4: the old warmup ended ~2.3us after the
#    first bigq chunk's completion semaphore, delaying the real stream.
#  - input DMA schedule: 6 bigq chunks (first 128 cols of BIGQ are dead — u0
#    starts at 8) + 2 xt halves, xt half 2 issued 4th so the mid-stream
#    diagonals (i>=32) never wait on it; 8 input DMAs total keeps every DMA on
#    a distinct completion semaphore (no reuse-serialization of late chunks).
#  - split output: psum cols 128:256 (t'=4..7) take their last matmul write at
#    diagonal i=66, so they are copied+DMAd while diagonals 67-70 still run;
#    only 64KB remains after the last matmul.
import os
import numpy as np

B = 32
NB = 512          # number of 16x16 blocks
NCORES = 8
MBLK = NB // NCORES   # 64 output block-rows per core
W = 576               # BIGQ window width (in u units of 16 columns)
ND = 71               # diagonal offsets d in [-63, 7]

DTYPE = "float16"
NWARM = int(os.environ.get("KNWARM", "2"))
SPLIT_OUT = os.environ.get("KSPLIT_OUT", "1") == "1"

_cached = {}
_last_results = None  # BassKernelResults of the most recent run (for profiling)


def _np_dtype(name):
    if name == "bfloat16":
        import ml_dtypes

        return ml_dtypes.bfloat16
    if name == "float16":
        return np.float16
    return np.float32


def _split_dt(dt_name):
    """'wt:mv' -> (weight dtype, moving dtype); single name -> same both."""
    if ":" in dt_name:
        wt, mv = dt_name.split(":")
        return wt, mv
    return dt_name, dt_name


def _build_program(dt_name):
    import concourse.bacc as bacc
    import concourse.mybir as mybir
    import concourse.tile as tile

    wt_name, mv_name = _split_dt(dt_name)
    wdt = getattr(mybir.dt, wt_name)
    mdt = getattr(mybir.dt, mv_name)
    f32 = mybir.dt.float32

    # Bacc (not plain Bass): its compile() pipeline splits multi-wait
    # instructions into EventSemaphore preludes (HW allows 1 wait/inst).
    nc = bacc.Bacc("TRN2", target_bir_lowering=False, debug=False, num_devices=NCORES)
    xt_d = nc.declare_dram_parameter("xt", [128, 2048], mdt, isOutput=False)
    bq_d = nc.declare_dram_parameter("bigq", [128, W * 16], wdt, isOutput=False)
    out_d = nc.declare_dram_parameter("out", [128, 256], f32, isOutput=True)

    with tile.TileContext(nc) as tc:
        with (
            tc.tile_pool(name="data", bufs=1) as data_pool,
            tc.tile_pool(name="psum", bufs=1, space="PSUM") as psum_pool,
        ):
            xt = data_pool.tile([128, 2048], mdt)
            bq = data_pool.tile([128, W * 16], wdt)
            out_sb = data_pool.tile([128, 256], f32)
            warm_sb = data_pool.tile([128, 256], f32)
            acc = psum_pool.tile([128, 256], f32)
            warm_ps = psum_pool.tile([128, 256], f32)

            # Input DMA schedule, in matmul consumption order. BIGQ cols
            # [0,128) are never read (stationary tiles start at u0=8), so the
            # bigq stream covers [128, 9216). Completion semaphores fire ~1us
            # after a chunk's last byte (HBM receipt round-trip), so the first
            # chunks are small (the stream's start is gated on bq[128:640] +
            # xt[0:512]) and later chunks grow. Alternate the two HWDGE rings
            # (sync=SP, scalar=ACT) so issue (~0.7us per DMA) overlaps.
            order = [
                (bq, bq_d, 128, 640),      # diagonals 0-3
                (xt, xt_d, 0, 512),        # xt cols for diagonals <= 14
                (bq, bq_d, 640, 1664),     # diagonals 4-11
                (bq, bq_d, 1664, 3200),    # diagonals 12-23
                (xt, xt_d, 512, 1024),     # diagonals <= 30
                (bq, bq_d, 3200, 4736),    # diagonals 24-35
                (xt, xt_d, 1024, 2048),    # diagonals >= 31
                (bq, bq_d, 4736, 6272),    # diagonals 36-47
                (bq, bq_d, 6272, 7808),    # diagonals 48-59
                (bq, bq_d, 7808, 9216),    # diagonals 60-70
            ]
            eng = [nc.sync, nc.scalar]
            for k, (tile_, dram_, lo, hi) in enumerate(order):
                eng[k % 2].dma_start(tile_[:, lo:hi], dram_[:, lo:hi])

            # PE warm-up while the first chunks stream in: the HAM clock gate
            # flips to 2.4GHz after ~3.4us of sustained PE activity measured
            # from warmup start, so what matters is starting the PE early;
            # the warmup only needs to bridge until the first chunk's
            # completion semaphore (~2us), not the full 3.4us. Each fp32
            # N=256 matmul is ~850ns cold.
            # memset on DVE, not gpsimd: DVE clears its engine preamble ~2us
            # earlier, so the warm-up (which waits on this) starts sooner.
            if NWARM:
                nc.vector.memset(warm_sb[:], 0.0)
            for wi in range(NWARM):
                nc.tensor.matmul(
                    warm_ps[:], warm_sb[:, 0:128], warm_sb[:],
                    start=(wi == 0), stop=(wi == NWARM - 1),
                )

            # d = t - c diagonal; stationary tile = BIGQ columns [16*u0, 16*u0+128)
            # with u0 = 8*i + 8 for i = 0..70 (d = i - 63).
            for i in range(ND):
                d = i - 63
                u0 = 8 * i + 8
                t_lo = max(0, d)
                t_hi = min(7, 63 + d)
                nt = t_hi - t_lo + 1
                tp_lo = 7 - t_hi           # flipped psum tile index
                cp_lo = 63 + d - t_hi      # reversed xt chunk index
                nc.tensor.matmul(
                    acc[:, 32 * tp_lo: 32 * (tp_lo + nt)],
                    bq[:, 16 * u0: 16 * u0 + 128],
                    xt[:, 32 * cp_lo: 32 * (cp_lo + nt)],
                    start=(i == 0),   # clears the whole PSUM bank
                    stop=(i == ND - 1),
                    skip_group_check=True,
                )

            if SPLIT_OUT:
                # psum cols 128:256 (t'=4..7) are final after diagonal 66
                # (its matmul writes cols 0:160; diagonals 67-70 write within
                # cols 0:128). Copy + store them while the tail runs, on
                # engines/rings disjoint from the final piece so the two
                # copy->issue->receipt chains overlap.
                nc.vector.tensor_copy(out_sb[:, 128:256], acc[:, 128:256])
                nc.sync.dma_start(out_d[:, 128:256], out_sb[:, 128:256])
                nc.vector.tensor_copy(out_sb[:, 0:128], acc[:, 0:128])
                nc.scalar.dma_start(out_d[:, 0:128], out_sb[:, 0:128])
            else:
                nc.vector.tensor_copy(out_sb[:], acc[:])
                nc.sync.dma_start(out_d[:], out_sb[:])
    nc.compile()
    return nc


def _get_program(dt_name):
    key = (dt_name, NWARM, SPLIT_OUT)
    if key not in _cached:
        _cached[key] = _build_program(dt_name)
    return _cached[key]


def _prep_inputs(x, blocks, dt_name):
    """Host-side layout prep (pure numpy reshuffles of the small inputs)."""
    x = np.ascontiguousarray(np.asarray(x), dtype=np.float32)
    blocks = np.ascontiguousarray(np.asarray(blocks), dtype=np.float32)
    # xt[(ni*16+q), c*32+b] = x[b, 128c + 16ni + q], then reverse c (c'=63-c)
    xt = x.T.reshape(64, 128, 32).transpose(1, 0, 2)[:, ::-1, :].reshape(128, 2048)
    xt = np.ascontiguousarray(xt)
    u = np.arange(W)
    ni = np.arange(8)
    wt_name, mv_name = _split_dt(dt_name)
    np_w, np_m = _np_dtype(wt_name), _np_dtype(mv_name)
    xt_c = np.ascontiguousarray(xt.astype(np_m))
    in_maps = []
    for k in range(NCORES):
        m0 = k * MBLK
        idx = (m0 + u[None, :] - ni[:, None]) % NB        # [8, W]
        bigq = blocks[idx]                                 # [8, W, p, q]
        bigq = bigq.transpose(0, 3, 1, 2).reshape(128, W * 16)  # [(ni,q), (u,p)]
        in_maps.append(
            {"xt": xt_c, "bigq": np.ascontiguousarray(bigq.astype(np_w))}
        )
    return in_maps


def _assemble(results):
    y = np.empty((B, NB * 16), dtype=np.float32)
    for k in range(NCORES):
        o = np.asarray(results[k]["out"])  # [128 (mi,p), 256 (t',b)], t = 7-t'
        y[:, 1024 * k: 1024 * (k + 1)] = (
            o.reshape(128, 8, 32)[:, ::-1, :].transpose(2, 1, 0).reshape(32, 1024)
        )
    return y


def kernel(x, blocks):
    global _last_results
    from concourse.bass_utils import run_bass_kernel_spmd

    nc = _get_program(DTYPE)
    in_maps = _prep_inputs(x, blocks, DTYPE)
    res = run_bass_kernel_spmd(nc, in_maps, list(range(NCORES)))
    _last_results = res
    return _assemble(res.results)
